# revision 24
# baseline (speedup 1.0000x reference)
"""Block attention (no softmax) Trainium2 Bass kernel, v2.

Problem: x:[8,8192,128] -> q,k,v projections -> per-256-block attention with
a +/-255-row K/V window, NO softmax, -> out:[8,8192,128].

Key algebraic identity: with no softmax, (Q K^T * s) V == (Q * s) (K^T V).
Per window n, M_n = sum_{r in win(n)} k_r v_r^T is a [128,128] matrix; then
out_blk = (Q_blk * s) @ M_n.

v2 structure (vs v1):
  * x ships from the host already TRANSPOSED ([d, s] fp16, contiguous) —
    no PE transposes, no DVE PSUM copies, big contiguous in-DMA runs.
  * K/V chunks are stored SHIFTED BY ONE ROW: chunk c holds rows
    128c+1 .. 128c+128 (projected from xT columns offset by +1 — free).
    Window n covers rows [256n-255, 256n+511) == shifted chunks
    2n-2..2n+2 full + chunk 2n+3 with K=126 — every chunk at base
    partition 0, so the v1 zeroed-row-0 GpSimd workaround disappears.
    Only window 0 needs a rank-1 k_0 v_0^T correction (row 0 has no home
    in the shifted layout).
  * Output computed TRANSPOSED: outT_n = M_n^T @ qT_block — one N=256
    matmul per window (stationary = M_n), and the out DMA is a contiguous
    [128, 8192] tensor (host un-transposes for free).

Sharding: batch (8) across the 8 NeuronCores, data-parallel.

All matmul operands fp16 (PSUM accumulates fp32; rel err ~4e-4 vs fp32).
"""

import sys
from contextlib import ExitStack

import numpy as np

for _p in ("/opt/trn_rl_repo", "/root/.axon_site/_ro/trn_rl_repo"):
    if _p not in sys.path:
        sys.path.append(_p)

import concourse.bass as bass
import concourse.tile as tile
from concourse import bacc, mybir
from concourse.bass_utils import run_bass_kernel_spmd

S = 8192          # sequence length per batch/core
D = 128           # input dim
H = 128           # hidden dim
BS = 256          # block size
HALO = 255        # window_size - 1
NB = S // BS      # 32 blocks
NP = NB // 2      # 16 window pairs
NCORES = 8
SCALE = float(1.0 / np.sqrt(np.float32(D)))

F32 = mybir.dt.float32
F16 = mybir.dt.float16
CDT = F16  # matmul operand dtype (PSUM accumulation is always fp32)
AF = mybir.ActivationFunctionType


def _window_chunks(n):
    """Shifted-layout chunks for window n: (chunk_idx, nrows) pairs.

    Shifted chunk c holds rows 128c+1 .. 128c+128 on partitions 0..127.
    Window n covers rows [max(0,256n-255), min(S,256n+511)); in shifted
    coords r' = r-1 that is chunks 2n-2..2n+2 full plus a K=126 tail
    (clipped at the sequence ends).  All chunks start at partition 0.
    """
    lo = max(0, BS * n - HALO) - 1   # shifted start (window 0: -1)
    hi = min(S, BS * n + BS + HALO) - 1  # shifted end (exclusive... inclusive r'-max is hi-1+... )
    # shifted r' range is [lo, hi) intersected with [0, S-1]  (r'=S-1 is the
    # padded row S which never appears because hi <= S-1+... )
    lo = max(0, lo)
    chunks = []
    a = lo
    while a < hi:
        b = min(hi, (a // 128 + 1) * 128)
        assert a % 128 == 0, (n, a)
        chunks.append((a // 128, b - a))
        a = b
    return chunks


def build_nc():
    nc = bacc.Bacc(
        "TRN2",
        target_bir_lowering=False,
        debug=False,
        enable_asserts=False,
        num_devices=NCORES,
    )

    xT = nc.dram_tensor("x", [D, S], CDT, kind="ExternalInput").ap()
    cw = nc.dram_tensor("cw", [128, 3 * H], CDT, kind="ExternalInput").ap()
    cb = nc.dram_tensor("cb", [128, 4 * H + 1], CDT, kind="ExternalInput").ap()
    out = nc.dram_tensor("out", [H, S], CDT, kind="ExternalOutput").ap()

    with ExitStack() as ctx:
        tc = ctx.enter_context(tile.TileContext(nc))
        const = ctx.enter_context(tc.tile_pool(name="const", bufs=1))
        cw_sb = const.tile([128, 3 * H], CDT)
        cb_sb = const.tile([128, 4 * H + 1], CDT)
        wq_sb = cw_sb[:, 0:H]
        wkv_sb = cw_sb[:, H : 3 * H]
        bkv_sb = cb_sb[:, 0 : 4 * H].rearrange("p (a b) -> p a b", a=2)
        bq_sb = cb_sb[:, 4 * H : 4 * H + 1]
        kv0_sb = const.tile([128, 2 * H], CDT)  # row 0 only: [k_0 | v_0]

        big = ctx.enter_context(tc.tile_pool(name="big", bufs=1))
        # xT with one zeroed spare column so shifted chunk 63's stationary
        # (columns 8065..8192) is a full 128 cols; col 8192 = 0 -> harmless.
        xT_sb = big.tile([128, S + 128], CDT)
        qT_all = big.tile([128, S], CDT)                 # q^T scaled, [h, s]
        kv_all = big.tile([128, S // 128, 2 * H], CDT)   # shifted [p, c, k|v]

        m_pool = ctx.enter_context(tc.tile_pool(name="m", bufs=4))
        o_pool = ctx.enter_context(tc.tile_pool(name="o", bufs=4))
        psum = ctx.enter_context(
            tc.tile_pool(name="ps", bufs=8, space=bass.MemorySpace.PSUM)
        )

        # ---- PE warm-up: HAM flips the PE clock 1.2->2.4 GHz only after
        # ~3.4us of sustained activity.  Burn dummy matmuls on scratch data
        # during the startup DMA window so every real matmul runs warm.
        warm_sb = const.tile([128, 512], CDT)
        nc.vector.memset(warm_sb, 0.0)
        psW = psum.tile([128, 512], F32, tag="ps", name="psW")
        # 8 cold N=512 matmuls ~= 3.4us: bridges the PE from t~8us until the
        # first x slice lands (~11us) with NO idle gap — a gap resets the
        # HAM activity window and the whole front of the kernel runs at
        # 1.2 GHz (measured +6us).
        for _ in range(8):
            nc.tensor.matmul(psW, warm_sb[:, 0:128], warm_sb, start=True, stop=True)
        nc.gpsimd.memset(xT_sb[:, S : S + 128], 0.0)

        # Each dma_start costs ~610ns of serial Sync-engine trigger time
        # (DIRECT2D) regardless of size, so ship x in only 3 slices and the
        # consts in 2.  Weights + first x slice gate the pipeline start;
        # biases are only needed once the first PSUM results drain.
        nc.sync.dma_start(cw_sb, cw)

        # ---- window-pair emitters -------------------------------------------
        m2_tiles = {}

        def emit_windows(t):
            """Accumulate M_n for windows 2t and 2t+1 into one PSUM bank.

            The two windows' accumulation groups stay sequential: start=True
            clears has_written bits for the whole bank, so groups in a shared
            bank must not interleave.  (PSUM tiles are padded to a full 2KB
            bank so no foreign tile can share the bank either.)
            """
            psM = psum.tile([128, 4, 128], F32, tag="ps", name="psM")
            for w in range(2):
                chunks = _window_chunks(2 * t + w)
                for i, (c, nr) in enumerate(chunks):
                    nc.tensor.matmul(
                        psM[:, w, :],
                        kv_all[0:nr, c, 0:H],
                        kv_all[0:nr, c, H : 2 * H],
                        start=(i == 0),
                        stop=(i == len(chunks) - 1) and not (t == 0 and w == 0),
                    )
                if t == 0 and w == 0:
                    # window 0: rank-1 correction for row 0 (absent from the
                    # shifted layout): psM[:,0,:] += k_0 v_0^T
                    nc.tensor.matmul(
                        psM[:, 0, :],
                        kv0_sb[0:1, 0:H],
                        kv0_sb[0:1, H : 2 * H],
                        start=False,
                        stop=True,
                    )
            m2 = m_pool.tile([128, 2, 128], CDT, tag="m")
            nc.scalar.copy(m2, psM[:, 0:2, :])
            m2_tiles[t] = m2

        def emit_out(t, eng=None):
            """outT for windows 2t, 2t+1: one N=256 matmul each
            (stationary = M_n), then copy+DMA [128, 512] fp16."""
            m2 = m2_tiles.pop(t)
            psOT = psum.tile([128, 512], F32, tag="ps", name="psOT")
            for w in range(2):
                s0 = 512 * t + 256 * w
                nc.tensor.matmul(
                    psOT[:, 256 * w : 256 * (w + 1)],
                    m2[:, w, :],
                    qT_all[:, s0 : s0 + 256],
                    start=True,
                    stop=True,
                )
            ostage = o_pool.tile([128, 512], CDT, tag="o")
            # balance PSUM->SBUF copies across ACT and DVE
            if eng is None:
                eng = "v" if t % 3 == 2 else "s"
            if eng == "sv":
                # tail: halve copy latency by splitting across both engines
                nc.scalar.copy(ostage[:, 0:256], psOT[:, 0:256])
                nc.vector.tensor_copy(ostage[:, 256:512], psOT[:, 256:512])
            elif eng == "v":
                nc.vector.tensor_copy(ostage, psOT)
            else:
                nc.scalar.copy(ostage, psOT)
            nc.sync.dma_start(out[:, 512 * t : 512 * t + 512], ostage)

        nc.sync.dma_start(xT_sb[:, 0:1024], xT[:, 0:1024])
        nc.sync.dma_start(cb_sb, cb)
        nc.sync.dma_start(xT_sb[:, 1024:4096], xT[:, 1024:4096])
        nc.sync.dma_start(xT_sb[:, 4096:S], xT[:, 4096:S])

        # ---- main software-pipelined loop: 512 seq rows per iteration -------
        for ci in range(S // 512):

            # q^T chunk: [h, 512] = Wq @ xT ; bias+scale fused on ACT copy
            psQ = psum.tile([128, 512], F32, tag="ps", name="psQ")
            nc.tensor.matmul(
                psQ, wq_sb, xT_sb[:, 512 * ci : 512 * (ci + 1)],
                start=True, stop=True,
            )
            nc.scalar.activation(
                qT_all[:, 512 * ci : 512 * (ci + 1)],
                psQ,
                AF.Identity,
                bias=bq_sb,
                scale=SCALE,
            )

            def kv_pair(h):
                # two shifted k|v chunks: [s128, 256] = xT_c.T @ [wk | wv]
                psKV = psum.tile([128, 2, 2 * H], F32, tag="ps", name="psKV")
                for j in range(2):
                    c = 4 * ci + 2 * h + j
                    nc.tensor.matmul(
                        psKV[:, j, :],
                        xT_sb[:, 128 * c + 1 : 128 * c + 129],
                        wkv_sb,
                        start=True,
                        stop=True,
                    )
                cc = 4 * ci + 2 * h
                nc.vector.tensor_add(kv_all[:, cc : cc + 2, :], psKV, bkv_sb)

            kv_pair(0)

            if ci == 0:
                # row 0 of k|v (unshifted) for the window-0 rank-1 fix
                psR = psum.tile([128, 512], F32, tag="ps", name="psR")
                nc.tensor.matmul(
                    psR[0:1, 0 : 2 * H], xT_sb[:, 0:1], wkv_sb,
                    start=True, stop=True,
                )
                nc.vector.tensor_add(
                    kv0_sb[0:1, :], psR[0:1, 0 : 2 * H], bkv_sb[0:1, 0, :]
                )

            # windows/out of earlier pairs keep the PE queue fed; pair ci-1
            # needs chunks up to 4ci+1, made by kv_pair(0) just above.
            if ci >= 1:
                emit_windows(ci - 1)

            kv_pair(1)

            if ci >= 2:
                emit_out(ci - 2)

        emit_windows(NP - 1)
        emit_out(NP - 2, eng="v")
        emit_out(NP - 1, eng="sv")

    nc.compile()
    return nc


_NC_CACHE = None


def _get_nc():
    global _NC_CACHE
    if _NC_CACHE is None:
        _NC_CACHE = build_nc()
    return _NC_CACHE


def _make_in_maps(inputs):
    x = np.asarray(inputs["x"], dtype=np.float32)
    Wq = np.asarray(inputs["Wq"], dtype=np.float32)
    Wk = np.asarray(inputs["Wk"], dtype=np.float32)
    Wv = np.asarray(inputs["Wv"], dtype=np.float32)
    bq = np.asarray(inputs["bq"], dtype=np.float32)
    bk = np.asarray(inputs["bk"], dtype=np.float32)
    bv = np.asarray(inputs["bv"], dtype=np.float32)

    wdt = np.float16 if CDT == F16 else np.float32
    bkv_row = np.concatenate([bk, bv])
    bkv_rep = np.broadcast_to(np.tile(bkv_row, 2)[None, :], (128, 4 * H))
    cw = np.concatenate([Wq.T, Wk.T, Wv.T], axis=1).astype(wdt)
    # ACT computes func(in*scale + bias), so the q bias ships pre-scaled
    cb = np.concatenate(
        [bkv_rep, (bq * SCALE).reshape(H, 1)], axis=1
    ).astype(wdt)

    shared = {
        "cw": np.ascontiguousarray(cw),
        "cb": np.ascontiguousarray(cb),
    }
    x16 = x.astype(np.float16) if CDT == F16 else x
    return [
        {"x": np.ascontiguousarray(x16[c].T), **shared} for c in range(NCORES)
    ]


def kernel(**inputs):
    nc = _get_nc()
    in_maps = _make_in_maps(inputs)
    res = run_bass_kernel_spmd(nc, in_maps, core_ids=list(range(NCORES)))
    return np.stack(
        [res.results[c]["out"].T for c in range(NCORES)], axis=0
    ).astype(np.float32)


def run_traced(inputs):
    """Like kernel() but with NTFF tracing; returns (out, BassKernelResults)."""
    nc = _get_nc()
    in_maps = _make_in_maps(inputs)
    res = run_bass_kernel_spmd(
        nc, in_maps, core_ids=list(range(NCORES)), trace=True
    )
    out = np.stack(
        [res.results[c]["out"].T for c in range(NCORES)], axis=0
    ).astype(np.float32)
    return out, res


# revision 25
# speedup vs baseline: 1.0204x; 1.0204x over previous
"""Block attention (no softmax) Trainium2 Bass kernel, v2.

Problem: x:[8,8192,128] -> q,k,v projections -> per-256-block attention with
a +/-255-row K/V window, NO softmax, -> out:[8,8192,128].

Key algebraic identity: with no softmax, (Q K^T * s) V == (Q * s) (K^T V).
Per window n, M_n = sum_{r in win(n)} k_r v_r^T is a [128,128] matrix; then
out_blk = (Q_blk * s) @ M_n.

v2 structure (vs v1):
  * x ships from the host already TRANSPOSED ([d, s] fp16, contiguous) —
    no PE transposes, no DVE PSUM copies, big contiguous in-DMA runs.
  * K/V chunks are stored SHIFTED BY ONE ROW: chunk c holds rows
    128c+1 .. 128c+128 (projected from xT columns offset by +1 — free).
    Window n covers rows [256n-255, 256n+511) == shifted chunks
    2n-2..2n+2 full + chunk 2n+3 with K=126 — every chunk at base
    partition 0, so the v1 zeroed-row-0 GpSimd workaround disappears.
    Only window 0 needs a rank-1 k_0 v_0^T correction (row 0 has no home
    in the shifted layout).
  * Output computed TRANSPOSED: outT_n = M_n^T @ qT_block — one N=256
    matmul per window (stationary = M_n), and the out DMA is a contiguous
    [128, 8192] tensor (host un-transposes for free).

Sharding: batch (8) across the 8 NeuronCores, data-parallel.

All matmul operands fp16 (PSUM accumulates fp32; rel err ~4e-4 vs fp32).
"""

import sys
from contextlib import ExitStack

import numpy as np

for _p in ("/opt/trn_rl_repo", "/root/.axon_site/_ro/trn_rl_repo"):
    if _p not in sys.path:
        sys.path.append(_p)

import concourse.bass as bass
import concourse.tile as tile
from concourse import bacc, mybir
from concourse.bass_utils import run_bass_kernel_spmd

S = 8192          # sequence length per batch/core
D = 128           # input dim
H = 128           # hidden dim
BS = 256          # block size
HALO = 255        # window_size - 1
NB = S // BS      # 32 blocks
NP = NB // 2      # 16 window pairs
NCORES = 8
SCALE = float(1.0 / np.sqrt(np.float32(D)))

F32 = mybir.dt.float32
F16 = mybir.dt.float16
CDT = F16  # matmul operand dtype (PSUM accumulation is always fp32)
AF = mybir.ActivationFunctionType


def _window_chunks(n):
    """Shifted-layout chunks for window n: (chunk_idx, nrows) pairs.

    Shifted chunk c holds rows 128c+1 .. 128c+128 on partitions 0..127.
    Window n covers rows [max(0,256n-255), min(S,256n+511)); in shifted
    coords r' = r-1 that is chunks 2n-2..2n+2 full plus a K=126 tail
    (clipped at the sequence ends).  All chunks start at partition 0.
    """
    lo = max(0, BS * n - HALO) - 1   # shifted start (window 0: -1)
    hi = min(S, BS * n + BS + HALO) - 1  # shifted end (exclusive... inclusive r'-max is hi-1+... )
    # shifted r' range is [lo, hi) intersected with [0, S-1]  (r'=S-1 is the
    # padded row S which never appears because hi <= S-1+... )
    lo = max(0, lo)
    chunks = []
    a = lo
    while a < hi:
        b = min(hi, (a // 128 + 1) * 128)
        assert a % 128 == 0, (n, a)
        chunks.append((a // 128, b - a))
        a = b
    return chunks


def build_nc():
    nc = bacc.Bacc(
        "TRN2",
        target_bir_lowering=False,
        debug=False,
        enable_asserts=False,
        num_devices=NCORES,
    )

    xT = nc.dram_tensor("x", [D, S], CDT, kind="ExternalInput").ap()
    cw = nc.dram_tensor("cw", [128, 3 * H], CDT, kind="ExternalInput").ap()
    cb = nc.dram_tensor("cb", [128, 4 * H + 1], CDT, kind="ExternalInput").ap()
    out = nc.dram_tensor("out", [H, S], CDT, kind="ExternalOutput").ap()

    with ExitStack() as ctx:
        tc = ctx.enter_context(tile.TileContext(nc))
        const = ctx.enter_context(tc.tile_pool(name="const", bufs=1))
        cw_sb = const.tile([128, 3 * H], CDT)
        cb_sb = const.tile([128, 4 * H + 1], CDT)
        wq_sb = cw_sb[:, 0:H]
        wkv_sb = cw_sb[:, H : 3 * H]
        bkv_sb = cb_sb[:, 0 : 4 * H].rearrange("p (a b) -> p a b", a=2)
        bq_sb = cb_sb[:, 4 * H : 4 * H + 1]
        kv0_sb = const.tile([128, 2 * H], CDT)  # row 0 only: [k_0 | v_0]

        big = ctx.enter_context(tc.tile_pool(name="big", bufs=1))
        # xT with one zeroed spare column so shifted chunk 63's stationary
        # (columns 8065..8192) is a full 128 cols; col 8192 = 0 -> harmless.
        xT_sb = big.tile([128, S + 128], CDT)
        qT_all = big.tile([128, S], CDT)                 # q^T scaled, [h, s]
        kv_all = big.tile([128, S // 128, 2 * H], CDT)   # shifted [p, c, k|v]

        m_pool = ctx.enter_context(tc.tile_pool(name="m", bufs=4))
        o_pool = ctx.enter_context(tc.tile_pool(name="o", bufs=4))
        psum = ctx.enter_context(
            tc.tile_pool(name="ps", bufs=8, space=bass.MemorySpace.PSUM)
        )

        # ---- PE warm-up: HAM flips the PE clock 1.2->2.4 GHz only after
        # ~3.4us of sustained activity.  Burn dummy matmuls on scratch data
        # during the startup DMA window so every real matmul runs warm.
        warm_sb = const.tile([128, 512], CDT)
        nc.vector.memset(warm_sb, 0.0)
        psW = psum.tile([128, 512], F32, tag="ps", name="psW")
        # 8 cold N=512 matmuls ~= 3.4us: bridges the PE from t~8us until the
        # first x slice lands (~11us) with NO idle gap — a gap resets the
        # HAM activity window and the whole front of the kernel runs at
        # 1.2 GHz (measured +6us).
        for _ in range(8):
            nc.tensor.matmul(psW, warm_sb[:, 0:128], warm_sb, start=True, stop=True)
        nc.gpsimd.memset(xT_sb[:, S : S + 128], 0.0)

        # Each dma_start costs ~610ns of serial Sync-engine trigger time
        # (DIRECT2D) regardless of size, so ship x in only 3 slices and the
        # consts in 2.  Weights + first x slice gate the pipeline start;
        # biases are only needed once the first PSUM results drain.
        nc.sync.dma_start(cw_sb, cw)

        # ---- window-pair emitters -------------------------------------------
        m2_tiles = {}

        def emit_windows(t):
            """Accumulate M_n for windows 2t and 2t+1 into one PSUM bank.

            The two windows' accumulation groups stay sequential: start=True
            clears has_written bits for the whole bank, so groups in a shared
            bank must not interleave.  (PSUM tiles are padded to a full 2KB
            bank so no foreign tile can share the bank either.)
            """
            psM = psum.tile([128, 4, 128], F32, tag="ps", name="psM")
            for w in range(2):
                chunks = _window_chunks(2 * t + w)
                for i, (c, nr) in enumerate(chunks):
                    nc.tensor.matmul(
                        psM[:, w, :],
                        kv_all[0:nr, c, 0:H],
                        kv_all[0:nr, c, H : 2 * H],
                        start=(i == 0),
                        stop=(i == len(chunks) - 1) and not (t == 0 and w == 0),
                    )
                if t == 0 and w == 0:
                    # window 0: rank-1 correction for row 0 (absent from the
                    # shifted layout): psM[:,0,:] += k_0 v_0^T
                    nc.tensor.matmul(
                        psM[:, 0, :],
                        kv0_sb[0:1, 0:H],
                        kv0_sb[0:1, H : 2 * H],
                        start=False,
                        stop=True,
                    )
            m2 = m_pool.tile([128, 2, 128], CDT, tag="m")
            nc.scalar.copy(m2, psM[:, 0:2, :])
            m2_tiles[t] = m2

        def emit_out(t, eng=None):
            """outT for windows 2t, 2t+1: one N=256 matmul each
            (stationary = M_n), then copy+DMA [128, 512] fp16."""
            m2 = m2_tiles.pop(t)
            psOT = psum.tile([128, 512], F32, tag="ps", name="psOT")
            for w in range(2):
                s0 = 512 * t + 256 * w
                nc.tensor.matmul(
                    psOT[:, 256 * w : 256 * (w + 1)],
                    m2[:, w, :],
                    qT_all[:, s0 : s0 + 256],
                    start=True,
                    stop=True,
                )
            ostage = o_pool.tile([128, 512], CDT, tag="o")
            # balance PSUM->SBUF copies across ACT and DVE
            if eng is None:
                eng = "v" if t % 3 == 2 else "s"
            if eng == "sv":
                # tail: halve copy latency by splitting across both engines
                nc.scalar.copy(ostage[:, 0:256], psOT[:, 0:256])
                nc.vector.tensor_copy(ostage[:, 256:512], psOT[:, 256:512])
            elif eng == "v":
                nc.vector.tensor_copy(ostage, psOT)
            else:
                nc.scalar.copy(ostage, psOT)
            nc.sync.dma_start(out[:, 512 * t : 512 * t + 512], ostage)

        nc.sync.dma_start(xT_sb[:, 0:1024], xT[:, 0:1024])
        nc.sync.dma_start(cb_sb, cb)
        nc.sync.dma_start(xT_sb[:, 1024:2048], xT[:, 1024:2048])
        nc.sync.dma_start(xT_sb[:, 2048:4096], xT[:, 2048:4096])
        nc.sync.dma_start(xT_sb[:, 4096:S], xT[:, 4096:S])

        # ---- main software-pipelined loop: 512 seq rows per iteration -------
        for ci in range(S // 512):

            # q^T chunk: [h, 512] = Wq @ xT ; bias+scale fused on ACT copy
            psQ = psum.tile([128, 512], F32, tag="ps", name="psQ")
            nc.tensor.matmul(
                psQ, wq_sb, xT_sb[:, 512 * ci : 512 * (ci + 1)],
                start=True, stop=True,
            )
            nc.scalar.activation(
                qT_all[:, 512 * ci : 512 * (ci + 1)],
                psQ,
                AF.Identity,
                bias=bq_sb,
                scale=SCALE,
            )

            def kv_pair(h):
                # two shifted k|v chunks: [s128, 256] = xT_c.T @ [wk | wv]
                psKV = psum.tile([128, 2, 2 * H], F32, tag="ps", name="psKV")
                for j in range(2):
                    c = 4 * ci + 2 * h + j
                    nc.tensor.matmul(
                        psKV[:, j, :],
                        xT_sb[:, 128 * c + 1 : 128 * c + 129],
                        wkv_sb,
                        start=True,
                        stop=True,
                    )
                cc = 4 * ci + 2 * h
                nc.vector.tensor_add(kv_all[:, cc : cc + 2, :], psKV, bkv_sb)

            kv_pair(0)

            if ci == 0:
                # row 0 of k|v (unshifted) for the window-0 rank-1 fix
                psR = psum.tile([128, 512], F32, tag="ps", name="psR")
                nc.tensor.matmul(
                    psR[0:1, 0 : 2 * H], xT_sb[:, 0:1], wkv_sb,
                    start=True, stop=True,
                )
                nc.vector.tensor_add(
                    kv0_sb[0:1, :], psR[0:1, 0 : 2 * H], bkv_sb[0:1, 0, :]
                )

            # windows/out of earlier pairs keep the PE queue fed; pair ci-1
            # needs chunks up to 4ci+1, made by kv_pair(0) just above.
            if ci >= 1:
                emit_windows(ci - 1)

            kv_pair(1)

            if ci >= 2:
                emit_out(ci - 2)

        emit_windows(NP - 1)
        emit_out(NP - 2, eng="v")
        emit_out(NP - 1, eng="sv")

    nc.compile()
    return nc


_NC_CACHE = None


def _get_nc():
    global _NC_CACHE
    if _NC_CACHE is None:
        _NC_CACHE = build_nc()
    return _NC_CACHE


def _make_in_maps(inputs):
    x = np.asarray(inputs["x"], dtype=np.float32)
    Wq = np.asarray(inputs["Wq"], dtype=np.float32)
    Wk = np.asarray(inputs["Wk"], dtype=np.float32)
    Wv = np.asarray(inputs["Wv"], dtype=np.float32)
    bq = np.asarray(inputs["bq"], dtype=np.float32)
    bk = np.asarray(inputs["bk"], dtype=np.float32)
    bv = np.asarray(inputs["bv"], dtype=np.float32)

    wdt = np.float16 if CDT == F16 else np.float32
    bkv_row = np.concatenate([bk, bv])
    bkv_rep = np.broadcast_to(np.tile(bkv_row, 2)[None, :], (128, 4 * H))
    cw = np.concatenate([Wq.T, Wk.T, Wv.T], axis=1).astype(wdt)
    # ACT computes func(in*scale + bias), so the q bias ships pre-scaled
    cb = np.concatenate(
        [bkv_rep, (bq * SCALE).reshape(H, 1)], axis=1
    ).astype(wdt)

    shared = {
        "cw": np.ascontiguousarray(cw),
        "cb": np.ascontiguousarray(cb),
    }
    x16 = x.astype(np.float16) if CDT == F16 else x
    return [
        {"x": np.ascontiguousarray(x16[c].T), **shared} for c in range(NCORES)
    ]


def kernel(**inputs):
    nc = _get_nc()
    in_maps = _make_in_maps(inputs)
    res = run_bass_kernel_spmd(nc, in_maps, core_ids=list(range(NCORES)))
    return np.stack(
        [res.results[c]["out"].T for c in range(NCORES)], axis=0
    ).astype(np.float32)


def run_traced(inputs):
    """Like kernel() but with NTFF tracing; returns (out, BassKernelResults)."""
    nc = _get_nc()
    in_maps = _make_in_maps(inputs)
    res = run_bass_kernel_spmd(
        nc, in_maps, core_ids=list(range(NCORES)), trace=True
    )
    out = np.stack(
        [res.results[c]["out"].T for c in range(NCORES)], axis=0
    ).astype(np.float32)
    return out, res


# revision 30
# speedup vs baseline: 1.2164x; 1.1921x over previous
"""Block attention (no softmax) Trainium2 Bass kernel, v2.

Problem: x:[8,8192,128] -> q,k,v projections -> per-256-block attention with
a +/-255-row K/V window, NO softmax, -> out:[8,8192,128].

Key algebraic identity: with no softmax, (Q K^T * s) V == (Q * s) (K^T V).
Per window n, M_n = sum_{r in win(n)} k_r v_r^T is a [128,128] matrix; then
out_blk = (Q_blk * s) @ M_n.

v2 structure (vs v1):
  * x ships from the host already TRANSPOSED ([d, s] fp16, contiguous) —
    no PE transposes, no DVE PSUM copies, big contiguous in-DMA runs.
  * K/V chunks are stored SHIFTED BY ONE ROW: chunk c holds rows
    128c+1 .. 128c+128 (projected from xT columns offset by +1 — free).
    Window n covers rows [256n-255, 256n+511) == shifted chunks
    2n-2..2n+2 full + chunk 2n+3 with K=126 — every chunk at base
    partition 0, so the v1 zeroed-row-0 GpSimd workaround disappears.
    Only window 0 needs a rank-1 k_0 v_0^T correction (row 0 has no home
    in the shifted layout).
  * Output computed TRANSPOSED: outT_n = M_n^T @ qT_block — one N=256
    matmul per window (stationary = M_n), and the out DMA is a contiguous
    [128, 8192] tensor (host un-transposes for free).

Sharding: batch (8) across the 8 NeuronCores, data-parallel.

All matmul operands fp16 (PSUM accumulates fp32; rel err ~4e-4 vs fp32).
"""

import sys
from contextlib import ExitStack

import numpy as np

for _p in ("/opt/trn_rl_repo", "/root/.axon_site/_ro/trn_rl_repo"):
    if _p not in sys.path:
        sys.path.append(_p)

import concourse.bass as bass
import concourse.tile as tile
from concourse import bacc, mybir
from concourse.bass_utils import run_bass_kernel_spmd

S = 8192          # sequence length per batch/core
D = 128           # input dim
H = 128           # hidden dim
BS = 256          # block size
HALO = 255        # window_size - 1
NB = S // BS      # 32 blocks
NP = NB // 2      # 16 window pairs
NCORES = 8
SCALE = float(1.0 / np.sqrt(np.float32(D)))

F32 = mybir.dt.float32
F16 = mybir.dt.float16
CDT = F16  # matmul operand dtype (PSUM accumulation is always fp32)
AF = mybir.ActivationFunctionType


def _window_chunks(n):
    """Shifted-layout chunks for window n: (chunk_idx, nrows) pairs.

    Shifted chunk c holds rows 128c+1 .. 128c+128 on partitions 0..127.
    Window n covers rows [max(0,256n-255), min(S,256n+511)); in shifted
    coords r' = r-1 that is chunks 2n-2..2n+2 full plus a K=126 tail
    (clipped at the sequence ends).  All chunks start at partition 0.
    """
    lo = max(0, BS * n - HALO) - 1   # shifted start (window 0: -1)
    hi = min(S, BS * n + BS + HALO) - 1  # shifted end (exclusive... inclusive r'-max is hi-1+... )
    # shifted r' range is [lo, hi) intersected with [0, S-1]  (r'=S-1 is the
    # padded row S which never appears because hi <= S-1+... )
    lo = max(0, lo)
    chunks = []
    a = lo
    while a < hi:
        b = min(hi, (a // 128 + 1) * 128)
        assert a % 128 == 0, (n, a)
        chunks.append((a // 128, b - a))
        a = b
    return chunks


def build_nc():
    nc = bacc.Bacc(
        "TRN2",
        target_bir_lowering=False,
        debug=False,
        enable_asserts=False,
        num_devices=NCORES,
    )

    xT = nc.dram_tensor("x", [D, S], CDT, kind="ExternalInput").ap()
    cw = nc.dram_tensor("cw", [128, 3 * H], CDT, kind="ExternalInput").ap()
    cb = nc.dram_tensor("cb", [128, 6 * H + 1], CDT, kind="ExternalInput").ap()
    out = nc.dram_tensor("out", [H, S], CDT, kind="ExternalOutput").ap()

    with ExitStack() as ctx:
        tc = ctx.enter_context(tile.TileContext(nc))
        const = ctx.enter_context(tc.tile_pool(name="const", bufs=1))
        cw_sb = const.tile([128, 3 * H], CDT)
        cb_sb = const.tile([128, 6 * H + 1], CDT)
        wq_sb = cw_sb[:, 0:H]
        wkv_sb = cw_sb[:, H : 3 * H]
        bkv_sb = cb_sb[:, 0 : 4 * H].rearrange("p (a b) -> p a b", a=2)
        bq_sb = cb_sb[:, 4 * H : 4 * H + 1]
        # host-precomputed [k_0 | v_0] row for the window-0 rank-1 fix
        kv0_sb = cb_sb[:, 4 * H + 1 : 6 * H + 1]

        big = ctx.enter_context(tc.tile_pool(name="big", bufs=1))
        # xT with one zeroed spare column so shifted chunk 63's stationary
        # (columns 8065..8192) is a full 128 cols; col 8192 = 0 -> harmless.
        xT_sb = big.tile([128, S + 128], CDT)
        qT_all = big.tile([128, S], CDT)                 # q^T scaled, [h, s]
        kv_all = big.tile([128, S // 128, 2 * H], CDT)   # shifted [p, c, k|v]

        m_pool = ctx.enter_context(tc.tile_pool(name="m", bufs=4))
        o_pool = ctx.enter_context(tc.tile_pool(name="o", bufs=4))
        psum = ctx.enter_context(
            tc.tile_pool(name="ps", bufs=8, space=bass.MemorySpace.PSUM)
        )

        # ---- PE warm-up: HAM flips the PE clock 1.2->2.4 GHz only after
        # ~3.4us of sustained activity.  Burn dummy matmuls on scratch data
        # during the startup DMA window so every real matmul runs warm.
        warm_sb = const.tile([128, 256], CDT)
        nc.gpsimd.memset(warm_sb, 0.0)
        psW = psum.tile([128, 512], F32, tag="ps", name="psW")
        # ~3.5us of cold matmuls bridges the PE from t~7us until the first x
        # slice lands (~11us) with NO idle gap — a gap resets the HAM
        # activity window and the whole front of the kernel runs at 1.2 GHz
        # (measured +6us).  N=256 granularity keeps the overrun small.
        for _ in range(16):
            nc.tensor.matmul(
                psW[:, 0:256], warm_sb[:, 0:128], warm_sb, start=True, stop=True
            )
        nc.gpsimd.memset(xT_sb[:, S : S + 128], 0.0)

        # Each dma_start costs ~610ns of serial Sync-engine trigger time
        # (DIRECT2D) regardless of size, so ship x in only 3 slices and the
        # consts in 2.  Weights + first x slice gate the pipeline start;
        # biases are only needed once the first PSUM results drain.
        nc.sync.dma_start(cw_sb, cw)

        # ---- window-pair emitters -------------------------------------------
        m2_tiles = {}

        def emit_windows(t):
            """Accumulate M_n for windows 2t and 2t+1 into one PSUM bank.

            The two windows' accumulation groups stay sequential: start=True
            clears has_written bits for the whole bank, so groups in a shared
            bank must not interleave.  (PSUM tiles are padded to a full 2KB
            bank so no foreign tile can share the bank either.)
            """
            psM = psum.tile([128, 4, 128], F32, tag="ps", name="psM")
            for w in range(2):
                chunks = _window_chunks(2 * t + w)
                for i, (c, nr) in enumerate(chunks):
                    nc.tensor.matmul(
                        psM[:, w, :],
                        kv_all[0:nr, c, 0:H],
                        kv_all[0:nr, c, H : 2 * H],
                        start=(i == 0),
                        stop=(i == len(chunks) - 1) and not (t == 0 and w == 0),
                    )
                if t == 0 and w == 0:
                    # window 0: rank-1 correction for row 0 (absent from the
                    # shifted layout): psM[:,0,:] += k_0 v_0^T
                    nc.tensor.matmul(
                        psM[:, 0, :],
                        kv0_sb[0:1, 0:H],
                        kv0_sb[0:1, H : 2 * H],
                        start=False,
                        stop=True,
                    )
            m2 = m_pool.tile([128, 2, 128], CDT, tag="m")
            nc.scalar.copy(m2, psM[:, 0:2, :])
            m2_tiles[t] = m2

        def emit_out(t, eng=None):
            """outT for windows 2t, 2t+1: one N=256 matmul each
            (stationary = M_n), then copy+DMA [128, 512] fp16."""
            m2 = m2_tiles.pop(t)
            psOT = psum.tile([128, 512], F32, tag="ps", name="psOT")
            for w in range(2):
                s0 = 512 * t + 256 * w
                nc.tensor.matmul(
                    psOT[:, 256 * w : 256 * (w + 1)],
                    m2[:, w, :],
                    qT_all[:, s0 : s0 + 256],
                    start=True,
                    stop=True,
                )
            ostage = o_pool.tile([128, 512], CDT, tag="o")
            # balance PSUM->SBUF copies across ACT and DVE
            if eng is None:
                eng = "v" if t % 3 == 2 else "s"
            if eng == "sv":
                # tail: halve copy latency by splitting across both engines
                nc.scalar.copy(ostage[:, 0:256], psOT[:, 0:256])
                nc.vector.tensor_copy(ostage[:, 256:512], psOT[:, 256:512])
            elif eng == "v":
                nc.vector.tensor_copy(ostage, psOT)
            else:
                nc.scalar.copy(ostage, psOT)
            nc.sync.dma_start(out[:, 512 * t : 512 * t + 512], ostage)

        nc.sync.dma_start(xT_sb[:, 0:1024], xT[:, 0:1024])
        nc.sync.dma_start(cb_sb, cb)
        nc.sync.dma_start(xT_sb[:, 1024:2048], xT[:, 1024:2048])
        nc.sync.dma_start(xT_sb[:, 2048:4096], xT[:, 2048:4096])
        nc.sync.dma_start(xT_sb[:, 4096:S], xT[:, 4096:S])

        # ---- main software-pipelined loop: 512 seq rows per iteration -------
        for ci in range(S // 512):

            # q^T chunk: [h, 512] = Wq @ xT ; bias+scale fused on ACT copy
            psQ = psum.tile([128, 512], F32, tag="ps", name="psQ")
            nc.tensor.matmul(
                psQ, wq_sb, xT_sb[:, 512 * ci : 512 * (ci + 1)],
                start=True, stop=True,
            )
            nc.scalar.activation(
                qT_all[:, 512 * ci : 512 * (ci + 1)],
                psQ,
                AF.Identity,
                bias=bq_sb,
                scale=SCALE,
            )

            def kv_pair(h):
                # two shifted k|v chunks: [s128, 256] = xT_c.T @ [wk | wv]
                psKV = psum.tile([128, 2, 2 * H], F32, tag="ps", name="psKV")
                for j in range(2):
                    c = 4 * ci + 2 * h + j
                    nc.tensor.matmul(
                        psKV[:, j, :],
                        xT_sb[:, 128 * c + 1 : 128 * c + 129],
                        wkv_sb,
                        start=True,
                        stop=True,
                    )
                cc = 4 * ci + 2 * h
                nc.vector.tensor_add(kv_all[:, cc : cc + 2, :], psKV, bkv_sb)

            kv_pair(0)

            # windows/out of earlier pairs keep the PE queue fed; pair ci-1
            # needs chunks up to 4ci+1, made by kv_pair(0) just above.
            if ci >= 1:
                emit_windows(ci - 1)

            kv_pair(1)

            if ci >= 2:
                emit_out(ci - 2)

        emit_windows(NP - 1)
        emit_out(NP - 2, eng="v")
        emit_out(NP - 1, eng="sv")

    nc.compile()
    return nc


_NC_CACHE = None


def _get_nc():
    global _NC_CACHE
    if _NC_CACHE is None:
        _NC_CACHE = build_nc()
    return _NC_CACHE


def _make_in_maps(inputs):
    x = np.asarray(inputs["x"], dtype=np.float32)
    Wq = np.asarray(inputs["Wq"], dtype=np.float32)
    Wk = np.asarray(inputs["Wk"], dtype=np.float32)
    Wv = np.asarray(inputs["Wv"], dtype=np.float32)
    bq = np.asarray(inputs["bq"], dtype=np.float32)
    bk = np.asarray(inputs["bk"], dtype=np.float32)
    bv = np.asarray(inputs["bv"], dtype=np.float32)

    wdt = np.float16 if CDT == F16 else np.float32
    bkv_row = np.concatenate([bk, bv])
    bkv_rep = np.broadcast_to(np.tile(bkv_row, 2)[None, :], (128, 4 * H))
    cw = np.concatenate([Wq.T, Wk.T, Wv.T], axis=1).astype(wdt)
    x16 = x.astype(np.float16) if CDT == F16 else x

    maps = []
    for c in range(NCORES):
        # host-precomputed row 0 of k|v (window-0 rank-1 fix), on partition 0
        kv0 = np.zeros((128, 2 * H), dtype=np.float32)
        kv0[0, :H] = x16[c, 0].astype(np.float32) @ Wk.T + bk
        kv0[0, H:] = x16[c, 0].astype(np.float32) @ Wv.T + bv
        # ACT computes func(in*scale + bias), so the q bias ships pre-scaled
        cb = np.concatenate(
            [bkv_rep, (bq * SCALE).reshape(H, 1), kv0], axis=1
        ).astype(wdt)
        maps.append(
            {
                "x": np.ascontiguousarray(x16[c].T),
                "cw": np.ascontiguousarray(cw),
                "cb": np.ascontiguousarray(cb),
            }
        )
    return maps


def kernel(**inputs):
    nc = _get_nc()
    in_maps = _make_in_maps(inputs)
    res = run_bass_kernel_spmd(nc, in_maps, core_ids=list(range(NCORES)))
    return np.stack(
        [res.results[c]["out"].T for c in range(NCORES)], axis=0
    ).astype(np.float32)


def run_traced(inputs):
    """Like kernel() but with NTFF tracing; returns (out, BassKernelResults)."""
    nc = _get_nc()
    in_maps = _make_in_maps(inputs)
    res = run_bass_kernel_spmd(
        nc, in_maps, core_ids=list(range(NCORES)), trace=True
    )
    out = np.stack(
        [res.results[c]["out"].T for c in range(NCORES)], axis=0
    ).astype(np.float32)
    return out, res


# revision 31
# speedup vs baseline: 1.2243x; 1.0065x over previous
"""Block attention (no softmax) Trainium2 Bass kernel, v2.

Problem: x:[8,8192,128] -> q,k,v projections -> per-256-block attention with
a +/-255-row K/V window, NO softmax, -> out:[8,8192,128].

Key algebraic identity: with no softmax, (Q K^T * s) V == (Q * s) (K^T V).
Per window n, M_n = sum_{r in win(n)} k_r v_r^T is a [128,128] matrix; then
out_blk = (Q_blk * s) @ M_n.

v2 structure (vs v1):
  * x ships from the host already TRANSPOSED ([d, s] fp16, contiguous) —
    no PE transposes, no DVE PSUM copies, big contiguous in-DMA runs.
  * K/V chunks are stored SHIFTED BY ONE ROW: chunk c holds rows
    128c+1 .. 128c+128 (projected from xT columns offset by +1 — free).
    Window n covers rows [256n-255, 256n+511) == shifted chunks
    2n-2..2n+2 full + chunk 2n+3 with K=126 — every chunk at base
    partition 0, so the v1 zeroed-row-0 GpSimd workaround disappears.
    Only window 0 needs a rank-1 k_0 v_0^T correction (row 0 has no home
    in the shifted layout).
  * Output computed TRANSPOSED: outT_n = M_n^T @ qT_block — one N=256
    matmul per window (stationary = M_n), and the out DMA is a contiguous
    [128, 8192] tensor (host un-transposes for free).

Sharding: batch (8) across the 8 NeuronCores, data-parallel.

All matmul operands fp16 (PSUM accumulates fp32; rel err ~4e-4 vs fp32).
"""

import sys
from contextlib import ExitStack

import numpy as np

for _p in ("/opt/trn_rl_repo", "/root/.axon_site/_ro/trn_rl_repo"):
    if _p not in sys.path:
        sys.path.append(_p)

import concourse.bass as bass
import concourse.tile as tile
from concourse import bacc, mybir
from concourse.bass_utils import run_bass_kernel_spmd

S = 8192          # sequence length per batch/core
D = 128           # input dim
H = 128           # hidden dim
BS = 256          # block size
HALO = 255        # window_size - 1
NB = S // BS      # 32 blocks
NP = NB // 2      # 16 window pairs
NCORES = 8
SCALE = float(1.0 / np.sqrt(np.float32(D)))

F32 = mybir.dt.float32
F16 = mybir.dt.float16
CDT = F16  # matmul operand dtype (PSUM accumulation is always fp32)
AF = mybir.ActivationFunctionType


def _window_chunks(n):
    """Shifted-layout chunks for window n: (chunk_idx, nrows) pairs.

    Shifted chunk c holds rows 128c+1 .. 128c+128 on partitions 0..127.
    Window n covers rows [max(0,256n-255), min(S,256n+511)); in shifted
    coords r' = r-1 that is chunks 2n-2..2n+2 full plus a K=126 tail
    (clipped at the sequence ends).  All chunks start at partition 0.
    """
    lo = max(0, BS * n - HALO) - 1   # shifted start (window 0: -1)
    hi = min(S, BS * n + BS + HALO) - 1  # shifted end (exclusive... inclusive r'-max is hi-1+... )
    # shifted r' range is [lo, hi) intersected with [0, S-1]  (r'=S-1 is the
    # padded row S which never appears because hi <= S-1+... )
    lo = max(0, lo)
    chunks = []
    a = lo
    while a < hi:
        b = min(hi, (a // 128 + 1) * 128)
        assert a % 128 == 0, (n, a)
        chunks.append((a // 128, b - a))
        a = b
    return chunks


def build_nc():
    nc = bacc.Bacc(
        "TRN2",
        target_bir_lowering=False,
        debug=False,
        enable_asserts=False,
        num_devices=NCORES,
    )

    xT = nc.dram_tensor("x", [D, S], CDT, kind="ExternalInput").ap()
    cw = nc.dram_tensor("cw", [128, 3 * H], CDT, kind="ExternalInput").ap()
    cb = nc.dram_tensor("cb", [128, 6 * H + 1], CDT, kind="ExternalInput").ap()
    out = nc.dram_tensor("out", [H, S], CDT, kind="ExternalOutput").ap()

    with ExitStack() as ctx:
        tc = ctx.enter_context(tile.TileContext(nc))
        const = ctx.enter_context(tc.tile_pool(name="const", bufs=1))
        cw_sb = const.tile([128, 3 * H], CDT)
        cb_sb = const.tile([128, 6 * H + 1], CDT)
        wq_sb = cw_sb[:, 0:H]
        wkv_sb = cw_sb[:, H : 3 * H]
        bkv_sb = cb_sb[:, 0 : 4 * H].rearrange("p (a b) -> p a b", a=2)
        bq_sb = cb_sb[:, 4 * H : 4 * H + 1]
        # host-precomputed [k_0 | v_0] row for the window-0 rank-1 fix
        kv0_sb = cb_sb[:, 4 * H + 1 : 6 * H + 1]

        big = ctx.enter_context(tc.tile_pool(name="big", bufs=1))
        # xT with one zeroed spare column so shifted chunk 63's stationary
        # (columns 8065..8192) is a full 128 cols; col 8192 = 0 -> harmless.
        xT_sb = big.tile([128, S + 128], CDT)
        qT_all = big.tile([128, S], CDT)                 # q^T scaled, [h, s]
        kv_all = big.tile([128, S // 128, 2 * H], CDT)   # shifted [p, c, k|v]

        m_pool = ctx.enter_context(tc.tile_pool(name="m", bufs=4))
        o_pool = ctx.enter_context(tc.tile_pool(name="o", bufs=4))
        psum = ctx.enter_context(
            tc.tile_pool(name="ps", bufs=8, space=bass.MemorySpace.PSUM)
        )

        # ---- PE warm-up: HAM flips the PE clock 1.2->2.4 GHz only after
        # ~3.4us of sustained activity.  Burn dummy matmuls on scratch data
        # during the startup DMA window so every real matmul runs warm.
        warm_sb = const.tile([128, 256], CDT)
        nc.gpsimd.memset(warm_sb, 0.0)
        psW = psum.tile([128, 512], F32, tag="ps", name="psW")
        # ~3.5us of cold matmuls bridges the PE from t~7us until the first x
        # slice lands (~11us) with NO idle gap — a gap resets the HAM
        # activity window and the whole front of the kernel runs at 1.2 GHz
        # (measured +6us).  N=256 granularity keeps the overrun small.
        for _ in range(16):
            nc.tensor.matmul(
                psW[:, 0:256], warm_sb[:, 0:128], warm_sb, start=True, stop=True
            )
        nc.gpsimd.memset(xT_sb[:, S : S + 128], 0.0)

        # Each dma_start costs ~610ns of serial Sync-engine trigger time
        # (DIRECT2D) regardless of size, so ship x in only 3 slices and the
        # consts in 2.  Weights + first x slice gate the pipeline start;
        # biases are only needed once the first PSUM results drain.
        nc.sync.dma_start(cw_sb, cw)

        # ---- window-pair emitters -------------------------------------------
        m2_tiles = {}

        def emit_windows(t):
            """Accumulate M_n for windows 2t and 2t+1 into one PSUM bank.

            The two windows' accumulation groups stay sequential: start=True
            clears has_written bits for the whole bank, so groups in a shared
            bank must not interleave.  (PSUM tiles are padded to a full 2KB
            bank so no foreign tile can share the bank either.)
            """
            psM = psum.tile([128, 4, 128], F32, tag="ps", name="psM")
            for w in range(2):
                chunks = _window_chunks(2 * t + w)
                for i, (c, nr) in enumerate(chunks):
                    nc.tensor.matmul(
                        psM[:, w, :],
                        kv_all[0:nr, c, 0:H],
                        kv_all[0:nr, c, H : 2 * H],
                        start=(i == 0),
                        stop=(i == len(chunks) - 1) and not (t == 0 and w == 0),
                    )
                if t == 0 and w == 0:
                    # window 0: rank-1 correction for row 0 (absent from the
                    # shifted layout): psM[:,0,:] += k_0 v_0^T
                    nc.tensor.matmul(
                        psM[:, 0, :],
                        kv0_sb[0:1, 0:H],
                        kv0_sb[0:1, H : 2 * H],
                        start=False,
                        stop=True,
                    )
            m2 = m_pool.tile([128, 2, 128], CDT, tag="m")
            nc.scalar.copy(m2, psM[:, 0:2, :])
            m2_tiles[t] = m2

        def emit_out(t, eng=None):
            """outT for windows 2t, 2t+1: one N=256 matmul each
            (stationary = M_n), then copy+DMA [128, 512] fp16."""
            m2 = m2_tiles.pop(t)
            psOT = psum.tile([128, 512], F32, tag="ps", name="psOT")
            for w in range(2):
                s0 = 512 * t + 256 * w
                nc.tensor.matmul(
                    psOT[:, 256 * w : 256 * (w + 1)],
                    m2[:, w, :],
                    qT_all[:, s0 : s0 + 256],
                    start=True,
                    stop=True,
                )
            ostage = o_pool.tile([128, 512], CDT, tag="o")
            # balance PSUM->SBUF copies across ACT and DVE
            if eng is None:
                eng = "v" if t % 3 == 2 else "s"
            if eng == "sv":
                # tail: halve copy latency by splitting across both engines
                nc.scalar.copy(ostage[:, 0:256], psOT[:, 0:256])
                nc.vector.tensor_copy(ostage[:, 256:512], psOT[:, 256:512])
            elif eng == "v":
                nc.vector.tensor_copy(ostage, psOT)
            else:
                nc.scalar.copy(ostage, psOT)
            nc.sync.dma_start(out[:, 512 * t : 512 * t + 512], ostage)

        # Slice boundaries sit at 512k+1 so each iteration's shifted chunks
        # (which read one column past 512k) depend only on earlier slices.
        nc.sync.dma_start(xT_sb[:, 0:1025], xT[:, 0:1025])
        nc.sync.dma_start(cb_sb, cb)
        nc.sync.dma_start(xT_sb[:, 1025:2049], xT[:, 1025:2049])
        nc.sync.dma_start(xT_sb[:, 2049:4097], xT[:, 2049:4097])
        nc.sync.dma_start(xT_sb[:, 4097:S], xT[:, 4097:S])

        # ---- main software-pipelined loop: 512 seq rows per iteration -------
        for ci in range(S // 512):

            # q^T chunk: [h, 512] = Wq @ xT ; bias+scale fused on ACT copy
            psQ = psum.tile([128, 512], F32, tag="ps", name="psQ")
            nc.tensor.matmul(
                psQ, wq_sb, xT_sb[:, 512 * ci : 512 * (ci + 1)],
                start=True, stop=True,
            )
            nc.scalar.activation(
                qT_all[:, 512 * ci : 512 * (ci + 1)],
                psQ,
                AF.Identity,
                bias=bq_sb,
                scale=SCALE,
            )

            def kv_pair(h):
                # two shifted k|v chunks: [s128, 256] = xT_c.T @ [wk | wv]
                psKV = psum.tile([128, 2, 2 * H], F32, tag="ps", name="psKV")
                for j in range(2):
                    c = 4 * ci + 2 * h + j
                    nc.tensor.matmul(
                        psKV[:, j, :],
                        xT_sb[:, 128 * c + 1 : 128 * c + 129],
                        wkv_sb,
                        start=True,
                        stop=True,
                    )
                cc = 4 * ci + 2 * h
                nc.vector.tensor_add(kv_all[:, cc : cc + 2, :], psKV, bkv_sb)

            kv_pair(0)

            # windows/out of earlier pairs keep the PE queue fed; pair ci-1
            # needs chunks up to 4ci+1, made by kv_pair(0) just above.
            if ci >= 1:
                emit_windows(ci - 1)

            kv_pair(1)

            if ci >= 2:
                emit_out(ci - 2)

        emit_windows(NP - 1)
        emit_out(NP - 2, eng="v")
        emit_out(NP - 1, eng="sv")

    nc.compile()
    return nc


_NC_CACHE = None


def _get_nc():
    global _NC_CACHE
    if _NC_CACHE is None:
        _NC_CACHE = build_nc()
    return _NC_CACHE


def _make_in_maps(inputs):
    x = np.asarray(inputs["x"], dtype=np.float32)
    Wq = np.asarray(inputs["Wq"], dtype=np.float32)
    Wk = np.asarray(inputs["Wk"], dtype=np.float32)
    Wv = np.asarray(inputs["Wv"], dtype=np.float32)
    bq = np.asarray(inputs["bq"], dtype=np.float32)
    bk = np.asarray(inputs["bk"], dtype=np.float32)
    bv = np.asarray(inputs["bv"], dtype=np.float32)

    wdt = np.float16 if CDT == F16 else np.float32
    bkv_row = np.concatenate([bk, bv])
    bkv_rep = np.broadcast_to(np.tile(bkv_row, 2)[None, :], (128, 4 * H))
    cw = np.concatenate([Wq.T, Wk.T, Wv.T], axis=1).astype(wdt)
    x16 = x.astype(np.float16) if CDT == F16 else x

    maps = []
    for c in range(NCORES):
        # host-precomputed row 0 of k|v (window-0 rank-1 fix), on partition 0
        kv0 = np.zeros((128, 2 * H), dtype=np.float32)
        kv0[0, :H] = x16[c, 0].astype(np.float32) @ Wk.T + bk
        kv0[0, H:] = x16[c, 0].astype(np.float32) @ Wv.T + bv
        # ACT computes func(in*scale + bias), so the q bias ships pre-scaled
        cb = np.concatenate(
            [bkv_rep, (bq * SCALE).reshape(H, 1), kv0], axis=1
        ).astype(wdt)
        maps.append(
            {
                "x": np.ascontiguousarray(x16[c].T),
                "cw": np.ascontiguousarray(cw),
                "cb": np.ascontiguousarray(cb),
            }
        )
    return maps


def kernel(**inputs):
    nc = _get_nc()
    in_maps = _make_in_maps(inputs)
    res = run_bass_kernel_spmd(nc, in_maps, core_ids=list(range(NCORES)))
    return np.stack(
        [res.results[c]["out"].T for c in range(NCORES)], axis=0
    ).astype(np.float32)


def run_traced(inputs):
    """Like kernel() but with NTFF tracing; returns (out, BassKernelResults)."""
    nc = _get_nc()
    in_maps = _make_in_maps(inputs)
    res = run_bass_kernel_spmd(
        nc, in_maps, core_ids=list(range(NCORES)), trace=True
    )
    out = np.stack(
        [res.results[c]["out"].T for c in range(NCORES)], axis=0
    ).astype(np.float32)
    return out, res
